# revision 25
# baseline (speedup 1.0000x reference)
"""AttentionalPooler Trainium2 kernel: 8-core data-parallel over batch.

Math restructuring (host side, exact algebra):
  - q = LN(queries)@Wq*scale is batch-independent -> precomputed on host, shipped
    transposed as qT[d, h*N+i] in bf16.
  - LN mean-subtraction folds into column-centered weight matrices:
      LN(x)@W = rstd * (x @ center(diag(g)W))   (center = subtract column means)
    Applied to Wkv (ctx LN) and W1 (post LN).
  - Softmax denominators S_h[i] come free from a ones-augmented v matmul
    (row 64 of each head's psum); they ride the head-pair evacuation into SBUF,
    are gathered by tiny DMAs, inverted with the fast custom-DVE reciprocal,
    expanded to [128, N] head-pair tiles by a K=8 one-hot selector matmul on
    the PE, and folded into an in-place multiply.
  - rstd of the ctx LN is applied via the Exp activation's per-partition scale
    (for k) and folded into the v psum->sbuf evacuation (for v).

v4 (PE density via software pipelining): the Tensor engine p-state only
reaches 2.4 GHz after ~3us of gapless execution, so the whole kernel is a
3-stage pipeline keeping the PE queue dense: while the Scalar engine chews
batch b's 16 Exp activations (~19us, the pacer), the PE runs batch b-1's
Wout matmuls; the post-LN sqrts of b-1 queue on Scalar AFTER b's exps (no
mid-phase activation-table thrash); the emb matmuls of b-2 interleave 1:1
into b's transposes so their LDWEIGHTS hide behind longer streams.  All
matmul traffic is bf16 (fp32 streams at 4 cycles/row and drew enough power
to DVFS-throttle the core).  Weight preloads ride the GpSimd DMA queue so
batch 0's x loads aren't stuck behind 5 MB of weights.
"""

import sys

sys.path.insert(0, "/opt/trn_rl_repo")

import numpy as np
import ml_dtypes

import concourse.bacc as bacc
import concourse.mybir as mybir
import concourse.tile as tile
from concourse.masks import make_identity

B, N, D = 32, 512, 1024
H, DH = 8, 64
INNER = H * DH  # 512
PROJ = 512
MID = (D + PROJ) // 2  # 768
EPS = 1e-5
NCORES = 8
BL = B // NCORES  # 4 batch items per core
FP = mybir.dt.float32
BF = mybir.dt.bfloat16
ACTF = mybir.ActivationFunctionType

P = 128
GELU_KIND = "gelu"  # "identity" for CoreSim (no Gelu in interpreter)
NJC = N // P  # 4 j-chunks
NCC = D // P  # 8 c-chunks
NIC = N // P  # 4 i-chunks
NINC = INNER // P  # 4 inner-chunks
NMC = MID // P  # 6 mid-chunks
NPC = PROJ // P  # 4 proj-chunks

BF_NP = ml_dtypes.bfloat16


def _host_prep(inputs):
    x = np.asarray(inputs["x"], np.float32)
    pos = np.asarray(inputs["pos_encoding"], np.float32)[0]  # [N, D]
    queries = np.asarray(inputs["queries"], np.float32)
    ln_q_g = np.asarray(inputs["ln_q_g"], np.float32)
    ln_ctx_g = np.asarray(inputs["ln_ctx_g"], np.float32)
    Wq = np.asarray(inputs["Wq"], np.float32)
    Wkv = np.asarray(inputs["Wkv"], np.float32)
    Wout = np.asarray(inputs["Wout"], np.float32)
    ln_post_g = np.asarray(inputs["ln_post_g"], np.float32)
    W1 = np.asarray(inputs["W1"], np.float32)
    b1 = np.asarray(inputs["b1"], np.float32)
    W2 = np.asarray(inputs["W2"], np.float32)
    b2 = np.asarray(inputs["b2"], np.float32)

    # Batch-independent query projection.
    qm = queries.mean(1, keepdims=True)
    qv = queries.var(1, keepdims=True)
    qn = (queries - qm) / np.sqrt(qv + EPS) * ln_q_g
    q = (qn @ Wq) * (DH ** -0.5)  # [N, INNER]
    # qT[d, h*N + i] = q[i, h*DH + d]
    qT = np.ascontiguousarray(
        q.reshape(N, H, DH).transpose(2, 1, 0).reshape(DH, H * N).astype(BF_NP)
    )

    Wg = ln_ctx_g[:, None] * Wkv
    wkv_c = np.ascontiguousarray((Wg - Wg.mean(0, keepdims=True)).astype(BF_NP))

    W1g = ln_post_g[:, None] * W1
    w1_c = np.ascontiguousarray(((W1g - W1g.mean(0, keepdims=True)) / N).astype(BF_NP))

    b1t = np.ascontiguousarray(b1.reshape(NMC, P).T)  # [128, 6]
    b2t = np.ascontiguousarray(b2.reshape(NPC, P).T)  # [128, 4]

    # e8[k, kc*128 + m] = 1 iff k == 2*kc + m//64 : selects 1/S rows for the
    # head-pair kc, upper/lower 64 partitions.
    e8 = np.zeros((8, NINC * P), np.float32)
    for kc in range(NINC):
        e8[2 * kc, kc * P : kc * P + DH] = 1.0
        e8[2 * kc + 1, kc * P + DH : (kc + 1) * P] = 1.0

    common = {
        "pos": np.ascontiguousarray(pos.astype(BF_NP)),
        "qT": qT,
        "wkv": wkv_c,
        "wout": np.ascontiguousarray(Wout.astype(BF_NP)),
        "w1": np.ascontiguousarray(w1_c),
        "b1t": b1t,
        "w2": np.ascontiguousarray(W2.astype(BF_NP)),
        "b2t": b2t,
        "e8": e8,
    }
    in_maps = []
    for c in range(NCORES):
        m = dict(common)
        m["x"] = np.ascontiguousarray(
            x[c * BL : (c + 1) * BL].reshape(BL * N, D).astype(BF_NP)
        )
        in_maps.append(m)
    return in_maps


def build_program():
    nc = bacc.Bacc("TRN2", target_bir_lowering=False, debug=False)
    x_d = nc.dram_tensor("x", [BL * N, D], BF, kind="ExternalInput")
    pos_d = nc.dram_tensor("pos", [N, D], BF, kind="ExternalInput")
    qT_d = nc.dram_tensor("qT", [DH, H * N], BF, kind="ExternalInput")
    wkv_d = nc.dram_tensor("wkv", [D, 2 * DH], BF, kind="ExternalInput")
    wout_d = nc.dram_tensor("wout", [INNER, D], BF, kind="ExternalInput")
    w1_d = nc.dram_tensor("w1", [D, MID], BF, kind="ExternalInput")
    b1t_d = nc.dram_tensor("b1t", [P, NMC], FP, kind="ExternalInput")
    w2_d = nc.dram_tensor("w2", [MID, PROJ], BF, kind="ExternalInput")
    b2t_d = nc.dram_tensor("b2t", [P, NPC], FP, kind="ExternalInput")
    e8_d = nc.dram_tensor("e8", [8, NINC * P], FP, kind="ExternalInput")
    out_d = nc.dram_tensor("predT", [PROJ, BL], FP, kind="ExternalOutput")

    from contextlib import ExitStack

    with tile.TileContext(nc) as tc, ExitStack() as ctx:
        pool = lambda name, bufs, **kw: ctx.enter_context(
            tc.tile_pool(name=name, bufs=bufs, **kw)
        )
        consts = pool("consts", 1)
        xraw_p = pool("xraw", 2)
        xnb_p = pool("xnb", 2)
        xT_p = pool("xT", 2)
        kv_p = pool("kv", 2)
        es_p = pool("es", 4)
        ov_p = pool("ov", 8)
        ost_p = pool("ost", 2)
        s8_p = pool("s8p", 2)
        onat_p = pool("onat", 1)
        small_p = pool("small", 2)
        # PSUM: 8 banks.  a={transpose pairs, sim head-pairs, MLP} (4KB x2 =
        # 4 banks), vo={kvps, vo, rb, embps} (2KB x2 = 2 banks),
        # wout={wp} (4KB x1 = 2 banks).
        ps_a = pool("ps_a", 2, space="PSUM")
        ps_vo = pool("ps_vo", 2, space="PSUM")
        ps_wout = pool("ps_wout", 1, space="PSUM")

        identb = consts.tile([P, P], BF)
        make_identity(nc, identb)
        eps_sb = consts.tile([P, 1], FP)
        nc.vector.memset(eps_sb[:, :], EPS)

        # Front-of-queue DMAs (needed by batch 0 immediately) on sync;
        # bulk weights on the gpsimd queue.
        # Queue layout: sync and scalar are the hardware DGE queues; gpsimd
        # issues software DGE.  Spread the startup-critical loads so batch 0
        # is fed in ~6us: x(b0) split by jc parity over sync/gpsimd, pos on
        # scalar (idle until the first sqrt), weights bulk-load on gpsimd
        # after x(b0).
        pos_sb = consts.tile([P, NJC * D], BF)
        for jc in range(NJC):
            nc.scalar.dma_start(
                pos_sb[:, jc * D : (jc + 1) * D], pos_d[jc * P : (jc + 1) * P, :]
            )
        wkv_sb = consts.tile([P, NCC * 2 * DH], BF)
        for cc in range(NCC):
            nc.sync.dma_start(
                wkv_sb[:, cc * 128 : (cc + 1) * 128],
                wkv_d[cc * P : (cc + 1) * P, :],
            )
        xr0 = xraw_p.tile([P, NJC * D], BF, tag="xr", name="xr0")
        for jc in range(NJC):
            eng = nc.sync if jc % 2 == 0 else nc.gpsimd
            eng.dma_start(
                xr0[:, jc * D : (jc + 1) * D], x_d[jc * P : (jc + 1) * P, :]
            )
        qT_sb = consts.tile([DH, H * N], BF)
        nc.sync.dma_start(qT_sb[:, :], qT_d[:, :])
        e8_sb = consts.tile([8, NINC * P], FP)
        nc.sync.dma_start(e8_sb[:, :], e8_d[:, :])
        e8b_sb = consts.tile([8, NINC * P], BF)
        nc.vector.tensor_copy(e8b_sb[:, :], e8_sb[:, :])
        b1t_sb = consts.tile([P, NMC], FP)
        nc.sync.dma_start(b1t_sb[:, :], b1t_d[:, :])
        b2t_sb = consts.tile([P, NPC], FP)
        nc.sync.dma_start(b2t_sb[:, :], b2t_d[:, :])
        wout_sb = consts.tile([P, NINC * D], BF)
        for kc in range(NINC):
            nc.gpsimd.dma_start(
                wout_sb[:, kc * D : (kc + 1) * D], wout_d[kc * P : (kc + 1) * P, :]
            )
        w1_sb = consts.tile([P, NCC * MID], BF)
        for cc in range(NCC):
            nc.gpsimd.dma_start(
                w1_sb[:, cc * MID : (cc + 1) * MID], w1_d[cc * P : (cc + 1) * P, :]
            )
        w2_sb = consts.tile([P, NMC * PROJ], BF)
        for mc in range(NMC):
            nc.gpsimd.dma_start(
                w2_sb[:, mc * PROJ : (mc + 1) * PROJ],
                w2_d[mc * P : (mc + 1) * P, :],
            )
        embT4 = consts.tile([P, BL * NCC], FP)  # col = b*NCC + cc
        h1_sb = consts.tile([P, NMC * BL], BF)
        pred_sb = consts.tile([P, NPC * BL], FP)

        # Per-b deferred state.
        ovs = {}  # b -> [4 ov tiles]
        onats = {}
        rstdobs = {}
        embpss = {}

        def emit_E1(bb, interleave=None, tail=False):
            """Wout matmuls + post-LN stats + onat evac for batch bb.
            Mid-pipeline (tail=False): scalar ops deferred to E2 so they queue
            behind the next batch's exps; psum from ps_wout.  Tail: the sim
            psum pool is idle, so wp double-buffers from ps_a, the rstd chain
            runs inline per ic, and batch bb's own emb matmuls interleave in
            as soon as their ic's rstd is ready."""
            interleave = interleave if interleave is not None else iter(())
            onat = onat_p.tile([P, NIC * D], BF, tag="onat", name=f"onat{bb}")
            onats[bb] = onat
            ag2 = small_p.tile([P, NIC, 2], FP, tag="bnag2", name=f"ag2_{bb}")
            own = iter(())
            if tail:
                sq2t = small_p.tile([P, NIC], FP, tag="sq2", name=f"sq2t_{bb}")
                rstdo = small_p.tile([P, NIC], FP, tag="rstdo", name=f"rstdo{bb}")
                rstdob = small_p.tile([P, NIC], BF, tag="rstdob", name=f"rstdob{bb}")
                rstdobs[bb] = rstdob
                own = emb_mms(bb)
            own_pulled = 0
            for ic in range(NIC):
                if tail:
                    wp = ps_a.tile([P, D], FP, tag="a", name=f"wpt{bb}_{ic}")
                else:
                    wp = ps_wout.tile([P, D], FP, tag="wout", name=f"wp{bb}_{ic}")
                for kc in range(NINC):
                    for half in range(2):
                        nc.tensor.matmul(
                            wp[:, half * 512 : (half + 1) * 512],
                            ovs[bb][kc][:, ic * P : (ic + 1) * P],
                            wout_sb[:, kc * D + half * 512 : kc * D + half * 512 + 512],
                            start=(kc == 0),
                            stop=(kc == NINC - 1),
                        )
                        mm = next(interleave, None)
                        if mm is not None:
                            mm()
                        if tail and own_pulled < ic * NCC:
                            mm2 = next(own, None)
                            if mm2 is not None:
                                mm2()
                                own_pulled += 1
                st2 = small_p.tile([P, 2, 6], FP, tag="bnst2", name=f"st2_{bb}_{ic}")
                for g in range(2):
                    nc.vector.bn_stats(st2[:, g, :], wp[:, g * 512 : (g + 1) * 512])
                nc.vector.bn_aggr(ag2[:, ic, :], st2[:, :, :])
                if tail:
                    nc.scalar.activation(
                        sq2t[:, ic : ic + 1],
                        ag2[:, ic, 1:2],
                        ACTF.Sqrt,
                        bias=eps_sb[:, :],
                    )
                    nc.vector.reciprocal(rstdo[:, ic : ic + 1], sq2t[:, ic : ic + 1])
                    nc.gpsimd.tensor_copy(
                        rstdob[:, ic : ic + 1], rstdo[:, ic : ic + 1]
                    )
                nc.vector.tensor_copy(onat[:, ic * D : (ic + 1) * D], wp[:, :])
            for mm in own:
                mm()
            return ag2

        def emit_E2(bb, ag2):
            """Scalar sqrts (queued after batch bb+1's exps) + recip + cast."""
            sq2 = small_p.tile([P, NIC], FP, tag="sq2", name=f"sq2_{bb}")
            rstdo = small_p.tile([P, NIC], FP, tag="rstdo", name=f"rstdo{bb}")
            rstdob = small_p.tile([P, NIC], BF, tag="rstdob", name=f"rstdob{bb}")
            rstdobs[bb] = rstdob
            for ic in range(NIC):
                nc.scalar.activation(
                    sq2[:, ic : ic + 1], ag2[:, ic, 1:2], ACTF.Sqrt, bias=eps_sb[:, :]
                )
            nc.vector.reciprocal(rstdo[:, :], sq2[:, :])
            nc.gpsimd.tensor_copy(rstdob[:, :], rstdo[:, :])

        def emb_mms(bb):
            """32 one-col emb matmuls for batch bb, as closures."""
            embps = ps_vo.tile([P, NIC, NCC], FP, tag="vo", name=f"embps{bb}")
            embpss[bb] = embps
            onat, rstdob = onats[bb], rstdobs[bb]
            for ic in range(NIC):
                for cc in range(NCC):
                    yield lambda ic=ic, cc=cc: nc.tensor.matmul(
                        embps[:, ic, cc : cc + 1],
                        onat[:, ic * D + cc * P : ic * D + (cc + 1) * P],
                        rstdob[:, ic : ic + 1],
                        start=True,
                        stop=True,
                    )

        def emit_embT4(bb):
            embps = embpss.pop(bb)
            ebt = embT4[:, bb * NCC : (bb + 1) * NCC]
            nc.vector.tensor_copy(ebt, embps[:, 0, :])
            for icp in range(1, NIC):
                nc.vector.tensor_add(ebt, ebt, embps[:, icp, :])

        for b in range(BL):
            # ---- A: load x (by column half), add pos -> bf16, ctx rstd ----
            xr = xr0 if b == 0 else xraw_p.tile([P, NJC * D], BF, tag="xr")
            xnb = xnb_p.tile([P, NJC * D], BF, tag="xnb")
            st4 = small_p.tile([P, NJC, 2, 6], FP, tag="bnst")
            for jc in range(NJC):
                s = slice(jc * D, (jc + 1) * D)
                if b > 0:
                    eng = nc.sync if jc % 2 == 0 else nc.gpsimd
                    eng.dma_start(
                        xr[:, s], x_d[b * N + jc * P : b * N + (jc + 1) * P, :]
                    )
                nc.vector.tensor_add(xnb[:, s], xr[:, s], pos_sb[:, s])
                for g in range(2):
                    nc.vector.bn_stats(
                        st4[:, jc, g, :],
                        xnb[:, jc * D + g * 512 : jc * D + (g + 1) * 512],
                    )
            rstd = small_p.tile([P, NJC], FP, tag="rstd")
            sq = small_p.tile([P, NJC], FP, tag="sq")
            for jc in range(NJC):
                ag = small_p.tile([P, 2], FP, tag="bnag")
                nc.vector.bn_aggr(ag[:, :], st4[:, jc, :, :])
                nc.scalar.activation(
                    sq[:, jc : jc + 1], ag[:, 1:2], ACTF.Sqrt, bias=eps_sb[:, :]
                )
            nc.vector.reciprocal(rstd[:, :], sq[:, :])

            # ---- B: transpose -> xT bf16, kv matmul; emb(b-2) interleaved --
            emb_it = emb_mms(b - 2) if b >= 2 else iter(())
            kvps = ps_wout.tile([P, N], FP, tag="wout", name=f"kvps{b}")
            for chalf in range(2):
                xT = xT_p.tile([P, 4 * N], BF, tag="xT")
                for ccp in range(2):
                    pt = ps_a.tile([P, 2 * N], BF, tag="a")
                    for cci in range(2):
                        cc = chalf * 4 + ccp * 2 + cci
                        for jc in range(NJC):
                            nc.tensor.transpose(
                                pt[:, cci * N + jc * P : cci * N + (jc + 1) * P],
                                xnb[:, jc * D + cc * P : jc * D + (cc + 1) * P],
                                identb[:, :],
                            )
                            mm = next(emb_it, None)
                            if mm is not None:
                                mm()
                    nc.vector.tensor_copy(
                        xT[:, ccp * 2 * N : (ccp + 1) * 2 * N], pt[:, :]
                    )
                for cc4 in range(4):
                    cc = chalf * 4 + cc4
                    nc.tensor.matmul(
                        kvps[:, :],
                        wkv_sb[:, cc * 128 : (cc + 1) * 128],
                        xT[:, cc4 * N : (cc4 + 1) * N],
                        start=(cc == 0),
                        stop=(cc == NCC - 1),
                    )
            kvT = kv_p.tile([P, N], BF, tag="kvT")
            nc.vector.tensor_copy(kvT[:, :], kvps[:, :])
            if b >= 2:
                emit_embT4(b - 2)

            # ---- v natural [j-part, d-free] bf16, scaled by rstd[j] ----
            v_nat = kv_p.tile([P, NJC * (DH + 1)], BF, tag="vnat")
            for jc in range(NJC):
                vt = ps_a.tile([P, DH], BF, tag="a")
                nc.tensor.transpose(
                    vt[:, :],
                    kvT[DH:, jc * P : (jc + 1) * P],
                    identb[DH:P, DH:P],
                )
                nc.vector.tensor_scalar_mul(
                    v_nat[:, jc * 65 : jc * 65 + DH],
                    vt[:, :],
                    rstd[:, jc : jc + 1],
                )
                nc.vector.memset(v_nat[:, jc * 65 + DH : (jc + 1) * 65], 1.0)

            # ---- C: sim + exp per (jc, head-pair); psum double-buffered ----
            es_tiles = []
            for jc in range(NJC):
                es_t = es_p.tile([P, H * N], BF, tag="es")
                es_tiles.append(es_t)
                for hq in range(4):
                    sm = ps_a.tile([P, 2 * N], FP, tag="a")
                    for hh in range(2):
                        h = hq * 2 + hh
                        nc.tensor.matmul(
                            sm[:, hh * N : (hh + 1) * N],
                            kvT[0:DH, jc * P : (jc + 1) * P],
                            qT_sb[:, h * N : (h + 1) * N],
                            start=True,
                            stop=True,
                        )
                    nc.scalar.activation(
                        es_t[:, hq * 2 * N : (hq + 1) * 2 * N],
                        sm[:, :],
                        ACTF.Exp,
                        scale=rstd[:, jc : jc + 1],
                    )

            # ---- E(b-1): wout on PE while scalar drains b's exps ----
            if b >= 1:
                ag2 = emit_E1(b - 1)
                emit_E2(b - 1, ag2)

            # ---- D: attn @ v in 2-head waves; S rides the evacuation ----
            ov4 = []
            ovs[b] = ov4
            s8 = s8_p.tile([8, N], BF, tag="s8")
            for w in range(4):
                vo0 = ps_vo.tile([DH + 1, N], FP, tag="vo")
                vo1 = ps_vo.tile([DH + 1, N], FP, tag="vo")
                for jc in range(NJC):
                    for hh, vo in ((0, vo0), (1, vo1)):
                        h = 2 * w + hh
                        nc.tensor.matmul(
                            vo[:, :],
                            v_nat[:, jc * 65 : (jc + 1) * 65],
                            es_tiles[jc][:, h * N : (h + 1) * N],
                            start=(jc == 0),
                            stop=(jc == NJC - 1),
                        )
                ov = ov_p.tile([P, N], BF, tag="ov", name=f"ov{b}_{w}")
                ov4.append(ov)
                # Even head + its S row (partition 64) in one copy.
                nc.vector.tensor_copy(ov[0 : DH + 1, :], vo0[:, :])
                nc.sync.dma_start(s8[2 * w : 2 * w + 1, :], ov[DH : DH + 1, :])
                ost = ost_p.tile([DH + 1, N], BF, tag="ost")
                nc.vector.tensor_copy(ost[:, :], vo1[:, :])
                nc.sync.dma_start(
                    s8[2 * w + 1 : 2 * w + 2, :], ost[DH : DH + 1, :]
                )
                # Odd v overwrites the S row region (queued after the S DMAs).
                nc.sync.dma_start(ov[DH:P, :], ost[0:DH, :])
            s8f = s8_p.tile([8, N], FP, tag="s8f")
            nc.vector.tensor_copy(s8f[:, :], s8[:, :])
            s8r = s8_p.tile([8, N], FP, tag="s8r")
            nc.vector.reciprocal_approx_fast(s8r[:, :], s8f[:, :])
            s8b = s8_p.tile([8, N], BF, tag="s8b")
            nc.vector.tensor_copy(s8b[:, :], s8r[:, :])
            for w in range(4):
                rb = ps_vo.tile([P, N], FP, tag="vo")
                nc.tensor.matmul(
                    rb[:, :],
                    e8b_sb[:, w * P : (w + 1) * P],
                    s8b[:, :],
                    start=True,
                    stop=True,
                )
                nc.vector.tensor_mul(ov4[w][:, :], ov4[w][:, :], rb[:, :])

        # ---- tail: E(3) with emb(2) and emb(3) interleaved inline, MLP ----
        emit_E1(BL - 1, interleave=emb_mms(BL - 2), tail=True)
        emit_embT4(BL - 2)
        emit_embT4(BL - 1)

        embT4b = consts.tile([P, BL * NCC], BF)
        nc.vector.tensor_copy(embT4b[:, :], embT4[:, :])
        embT4_r = embT4b.rearrange("p (b c) -> p c b", c=NCC)
        for mc in range(NMC):
            hp = ps_a.tile([P, BL], FP, tag="a")
            for cc in range(NCC):
                nc.tensor.matmul(
                    hp[:, :],
                    w1_sb[:, cc * MID + mc * P : cc * MID + (mc + 1) * P],
                    embT4_r[:, cc, :],
                    start=(cc == 0),
                    stop=(cc == NCC - 1),
                )
            gf = ACTF.Gelu if GELU_KIND == "gelu" else ACTF.Identity
            nc.scalar.activation(
                h1_sb[:, mc * BL : (mc + 1) * BL],
                hp[:, :],
                gf,
                bias=b1t_sb[:, mc : mc + 1],
            )
        for pc in range(NPC):
            pp = ps_a.tile([P, BL], FP, tag="a")
            for mc in range(NMC):
                nc.tensor.matmul(
                    pp[:, :],
                    w2_sb[:, mc * PROJ + pc * P : mc * PROJ + (pc + 1) * P],
                    h1_sb[:, mc * BL : (mc + 1) * BL],
                    start=(mc == 0),
                    stop=(mc == NMC - 1),
                )
            nc.vector.tensor_scalar_add(
                pred_sb[:, pc * BL : (pc + 1) * BL], pp[:, :], b2t_sb[:, pc : pc + 1]
            )
            nc.sync.dma_start(
                out_d[pc * P : (pc + 1) * P, :], pred_sb[:, pc * BL : (pc + 1) * BL]
            )

    nc.compile()
    return nc


_NC_CACHE = None


def kernel(**inputs) -> np.ndarray:
    global _NC_CACHE
    from concourse.bass_utils import run_bass_kernel_spmd

    in_maps = _host_prep(inputs)
    if _NC_CACHE is None:
        _NC_CACHE = build_program()
    nc = _NC_CACHE
    res = run_bass_kernel_spmd(nc, in_maps, core_ids=list(range(NCORES)))
    out = np.empty((B, PROJ), np.float32)
    for c in range(NCORES):
        out[c * BL : (c + 1) * BL] = res.results[c]["predT"].T
    return out
